# revision 1
# baseline (speedup 1.0000x reference)
"""Trainium2 Bass kernel for a 3-layer ResGatedGraphConv GNN (ClinicalGatedGCN).

Strategy (8 NeuronCores, SPMD):
  - Nodes are partitioned into 8 contiguous ranges (rank-blocked ids, padded to
    a multiple of 128 per rank). Edges are assigned to the rank that owns their
    dst node and sorted by (src-half, dst) on the host.
  - Each rank computes the full q/v node tables (node-major, directly from
    matmuls with the hT chunk as the stationary operand) into local HBM
    tables (split at row 32768 so int16 dma_gather indices reach every row in
    two epochs), plus a rank-local k table indexed by dst. Per edge, one
    dma_gather fetches the src row of [q|v] and one fetches k[dst].
  - Gate math (sigmoid(k[dst]+q[src]+attr*We)) runs on DVE/ACT in edge-major
    layout; segment-sum over dst is a PE matmul against a 0/1 selector matrix
    built on-device with is_equal (edges sorted by dst; 128-node groups
    accumulate in PSUM).
  - h stays feature-major; per layer the updated h slice is AllGather'd so the
    next layer's (replicated) table matmuls can see all nodes.
  - Mean-pool per graph is a matmul against a host-built indicator with 1/cnt
    folded in; partial pools are AllGather'd and summed; the tiny classifier
    runs on every core.
"""

import numpy as np
import ml_dtypes

import concourse.bacc as bacc
import concourse.bass as bass
import concourse.mybir as mybir
import concourse.tile as tile
from concourse.bass_utils import run_bass_kernel_spmd
from concourse.masks import make_identity

F32 = mybir.dt.float32
BF16 = mybir.dt.bfloat16
I16 = mybir.dt.int16
AF = mybir.ActivationFunctionType
OP = mybir.AluOpType

# ---------------- problem constants (hardcoded per spec) ----------------
N, E, H, G, NCLIN, NCLS = 50000, 800000, 128, 64, 16, 2
NLAYER = 3
EPS = 1e-5
SLOPE = 0.01
R = 8                      # ranks / NeuronCores
SPLIT = 32768              # int16 gather index limit -> 2 epochs

USE_BF16 = True            # table/h/gate dtype

NPR = (N + R - 1) // R     # real nodes per rank
NGRP = (NPR + 127) // 128  # 128-node groups per rank
NPAD = NGRP * 128          # padded nodes per rank
NTOT = R * NPAD            # rank-blocked total rows
CG = 1                     # groups per gather chunk


def _np_dtab(use_bf16):
    return ml_dtypes.bfloat16 if use_bf16 else np.float32


def wrap_idxs_block(idx):
    """Wrap one gather call's indices: idx j -> [j%16, j//16], tiled to 128 parts."""
    n = len(idx)
    assert n % 16 == 0
    w = np.asarray(idx, np.int16).reshape(n // 16, 16).T
    return np.tile(w, (8, 1))


def colmaj128(v):
    """Edge-scalar array -> [128, n/128] layout (edge j at [j%128, j//128])."""
    v = np.asarray(v)
    n = v.shape[0]
    assert n % 128 == 0
    return v.reshape(n // 128, 128).T.copy()


# ---------------------------------------------------------------------------
# host-side preprocessing
# ---------------------------------------------------------------------------

def prep(inputs, use_bf16=None):
    if use_bf16 is None:
        use_bf16 = USE_BF16
    dtab = _np_dtab(use_bf16)
    x = np.asarray(inputs["x"], np.float32)
    edge_index = np.asarray(inputs["edge_index"])
    edge_attr = np.asarray(inputs["edge_attr"], np.float32)[:, 0]
    batch = np.asarray(inputs["batch"]).astype(np.int64)
    clinical = np.asarray(inputs["clinical"], np.float32)
    Wk, bk = np.asarray(inputs["Wk"], np.float32), np.asarray(inputs["bk"], np.float32)
    Wq, bq = np.asarray(inputs["Wq"], np.float32), np.asarray(inputs["bq"], np.float32)
    Wv, bv = np.asarray(inputs["Wv"], np.float32), np.asarray(inputs["bv"], np.float32)
    Ws, bs = np.asarray(inputs["Ws"], np.float32), np.asarray(inputs["bs"], np.float32)
    We, be = np.asarray(inputs["We"], np.float32), np.asarray(inputs["be"], np.float32)
    gamma = np.asarray(inputs["gamma"], np.float32)
    beta = np.asarray(inputs["beta"], np.float32)
    rmean = np.asarray(inputs["rmean"], np.float32)
    rvar = np.asarray(inputs["rvar"], np.float32)
    Wc, bc = np.asarray(inputs["Wc"], np.float32), np.asarray(inputs["bc"], np.float32)

    src = edge_index[0].astype(np.int64)
    dst = edge_index[1].astype(np.int64)

    # BN folded: A*x + B
    A = gamma / np.sqrt(rvar + EPS)
    B = beta - rmean * A
    bgate = bk + bq + be          # folded into k table
    # rank-blocked row id
    rb_row = (src // NPR) * NPAD + (src % NPR)

    e_rank = dst // NPR
    epoch = (rb_row >= SPLIT).astype(np.int64)
    dst_local = dst - e_rank * NPR
    group = dst_local // 128
    dst_rel = dst_local % 128

    # per (rank, epoch, group) counts -> capacities (uniform across ranks)
    caps = [0, 0]
    counts = {}
    for ep in (0, 1):
        cnt = np.zeros((R, NGRP), np.int64)
        m = epoch == ep
        np.add.at(cnt, (e_rank[m], group[m]), 1)
        counts[ep] = cnt
        caps[ep] = max(1, int(np.ceil(cnt.max() / 128)))
    T0, T1 = caps

    # graph counts for mean pooling
    cntg = np.bincount(batch, minlength=G).astype(np.float32)
    inv_cnt = 1.0 / np.maximum(cntg, 1.0)

    # per-rank arrays
    order = np.lexsort((dst, group, epoch, e_rank))  # by rank, epoch, group, dst
    src_s, dst_rel_s, attr_s = rb_row[order], dst_rel[order], edge_attr[order]
    ep_s, rank_s, grp_s = epoch[order], e_rank[order], group[order]
    # start offset of each (rank, epoch, group) run in the sorted arrays
    key = ((rank_s * 2 + ep_s) * NGRP + grp_s)
    starts = np.searchsorted(key, np.arange(R * 2 * NGRP))

    in_maps = []
    iota_rep = np.tile(np.arange(128, dtype=np.float32), (128, 1))
    We_rep = np.stack([np.tile(We[l, 0], (128, 1)) for l in range(NLAYER)])
    bias_qv = np.zeros((NLAYER, 128, 2 * H), np.float32)
    bias_k = np.zeros((NLAYER, 128, H), np.float32)
    for l in range(NLAYER):
        bias_k[l, :, :] = bgate[l][None, :]
        bias_qv[l, :, H:2 * H] = bv[l][None, :]
    has_bias_qv = bool(np.any(bias_qv != 0))
    has_bias_k = bool(np.any(bias_k != 0))
    has_bs = bool(np.any(bs != 0))
    has_bc = bool(np.any(bc != 0))

    x_rb = np.zeros((R * 128, NPAD), np.float32)
    for r in range(R):
        lo, hi = r * NPR, min((r + 1) * NPR, N)
        x_rb[r * 128:(r + 1) * 128, 0:hi - lo] = x[lo:hi].T

    for r in range(R):
        ep_arrs = {}
        for ep, T in ((0, T0), (1, T1)):
            ntile = NGRP * T
            gidx = np.zeros((128, ntile * 8), np.int16)
            kidx = np.zeros((128, ntile * 8), np.int16)
            dstc = np.full((ntile * 128,), -1.0, np.float32)
            attrc = np.zeros((ntile * 128,), np.float32)
            cnt = counts[ep]
            for g in range(NGRP):
                k = (r * 2 + ep) * NGRP + g
                s0 = starts[k]
                n = cnt[r, g]
                e0 = g * T * 128
                dstc[e0:e0 + n] = dst_rel_s[s0:s0 + n]
                attrc[e0:e0 + n] = attr_s[s0:s0 + n]
                idx = np.zeros((T * 128,), np.int64)
                idx[:n] = src_s[s0:s0 + n] - ep * SPLIT
                gidx[:, g * T * 8:(g + 1) * T * 8] = wrap_idxs_block(idx)
                kdx = np.zeros((T * 128,), np.int64)
                kdx[:n] = g * 128 + dst_rel_s[s0:s0 + n]
                kidx[:, g * T * 8:(g + 1) * T * 8] = wrap_idxs_block(kdx)
            ep_arrs[ep] = (gidx, colmaj128(dstc).astype(dtab), colmaj128(attrc).astype(dtab), kidx)
        # pooling indicator with 1/cnt folded
        IndT = np.zeros((NPAD, G), np.float32)
        lo, hi = r * NPR, min((r + 1) * NPR, N)
        IndT[np.arange(hi - lo), batch[lo:hi]] = inv_cnt[batch[lo:hi]]
        im = {
            "x_rb": x_rb.astype(dtab),
            "xT_loc": x_rb[r * 128:(r + 1) * 128].astype(dtab),
            "Wk": Wk.astype(dtab), "Wq": Wq.astype(dtab), "Wv": Wv.astype(dtab),
            "Ws": Ws.astype(dtab),
            "We_rep": We_rep.astype(dtab),
            "bias_qv": bias_qv,
            "bias_k": bias_k,
            "bs_col": bs.reshape(NLAYER, H, 1),
            "A_col": A.reshape(NLAYER, H, 1),
            "B_col": B.reshape(NLAYER, H, 1),
            "iota_rep": iota_rep.astype(dtab),
            "gidx0": ep_arrs[0][0], "dst0": ep_arrs[0][1], "attr0": ep_arrs[0][2],
            "gidx1": ep_arrs[1][0], "dst1": ep_arrs[1][1], "attr1": ep_arrs[1][2],
            "kidx0": ep_arrs[0][3], "kidx1": ep_arrs[1][3],
            "IndT": IndT.astype(dtab),
            "clinT": clinical.T.copy(),
            "Wc_h": Wc[0:H], "Wc_c": Wc[H:H + NCLIN],
            "bc_rep": np.tile(bc, (G, 1)),
        }
        in_maps.append(im)
    meta = dict(T0=T0, T1=T1, has_bias_qv=has_bias_qv, has_bias_k=has_bias_k,
                has_bs=has_bs, has_bc=has_bc, use_bf16=use_bf16)
    return in_maps, meta


# ---------------------------------------------------------------------------
# device program
# ---------------------------------------------------------------------------

def build(meta):
    T0, T1 = meta["T0"], meta["T1"]
    use_bf16 = meta["use_bf16"]
    parts = meta.get("parts", 4)
    DT = BF16 if use_bf16 else F32

    nc = bacc.Bacc("TRN2", target_bir_lowering=False, debug=False, num_devices=R)

    def din(name, shape, dt):
        return nc.dram_tensor(name, shape, dt, kind="ExternalInput").ap()

    t_x_rb = din("x_rb", [R * 128, NPAD], DT)
    t_xT_loc = din("xT_loc", [128, NPAD], DT)
    t_Wk = din("Wk", [NLAYER, H, H], DT)
    t_Wq = din("Wq", [NLAYER, H, H], DT)
    t_Wv = din("Wv", [NLAYER, H, H], DT)
    t_Ws = din("Ws", [NLAYER, H, H], DT)
    t_We = din("We_rep", [NLAYER, 128, H], DT)
    t_bias_qv = din("bias_qv", [NLAYER, 128, 2 * H], F32)
    t_bias_k = din("bias_k", [NLAYER, 128, H], F32)
    t_bs = din("bs_col", [NLAYER, H, 1], F32)
    t_A = din("A_col", [NLAYER, H, 1], F32)
    t_B = din("B_col", [NLAYER, H, 1], F32)
    t_iota = din("iota_rep", [128, 128], DT)
    t_gidx = [din("gidx0", [128, NGRP * T0 * 8], I16),
              din("gidx1", [128, NGRP * T1 * 8], I16)]
    t_kidx = [din("kidx0", [128, NGRP * T0 * 8], I16),
              din("kidx1", [128, NGRP * T1 * 8], I16)]
    t_dst = [din("dst0", [128, NGRP * T0], DT),
             din("dst1", [128, NGRP * T1], DT)]
    t_attr = [din("attr0", [128, NGRP * T0], DT),
              din("attr1", [128, NGRP * T1], DT)]
    t_IndT = din("IndT", [NPAD, G], DT)
    t_clinT = din("clinT", [NCLIN, G], F32)
    t_Wc_h = din("Wc_h", [H, NCLS], F32)
    t_Wc_c = din("Wc_c", [NCLIN, NCLS], F32)
    t_bc = din("bc_rep", [G, NCLS], F32)

    t_out = nc.dram_tensor("out", [G, NCLS], F32, kind="ExternalOutput").ap()

    qv_lo = nc.dram_tensor("qv_lo", [SPLIT, 2 * H], DT).ap()
    qv_hi = nc.dram_tensor("qv_hi", [NTOT - SPLIT, 2 * H], DT).ap()
    k_loc = nc.dram_tensor("k_loc", [NPAD, H], DT).ap()
    h_loc = [nc.dram_tensor(f"h_loc{l}", [128, NPAD], DT).ap() for l in range(2)]
    ag_out = [nc.dram_tensor(f"ag_out{l}", [R * 128, NPAD], DT,
                             addr_space="Shared").ap() for l in range(2)]
    pool_in = nc.dram_tensor("pool_in", [G, H], F32).ap()
    pool_out = nc.dram_tensor("pool_out", [R * G, H], F32, addr_space="Shared").ap()

    # node-chunk structure along NPAD
    chunks = []
    c0 = 0
    while c0 < NPAD:
        csz = min(512, NPAD - c0)
        chunks.append((c0, csz))
        c0 += csz

    with tile.TileContext(nc) as tc:
        import contextlib
        with contextlib.ExitStack() as ctx:
            consts = ctx.enter_context(tc.tile_pool(name="consts", bufs=1))
            hsb = ctx.enter_context(tc.tile_pool(name="hsb", bufs=1))
            h3p = ctx.enter_context(tc.tile_pool(name="h3p", bufs=1))
            lhp = ctx.enter_context(tc.tile_pool(name="lhp", bufs=4))
            stg = ctx.enter_context(tc.tile_pool(name="stg", bufs=4))
            edg = ctx.enter_context(tc.tile_pool(name="edg", bufs=4))
            edm = ctx.enter_context(tc.tile_pool(name="edm", bufs=4))
            pnode = ctx.enter_context(tc.tile_pool(name="pnode", bufs=4, space="PSUM"))
            pedge = ctx.enter_context(tc.tile_pool(name="pedge", bufs=3, space="PSUM"))
            ppool = ctx.enter_context(tc.tile_pool(name="ppool", bufs=1, space="PSUM"))

            _cid = [0]

            def load_const(src_ap, shape, dt):
                _cid[0] += 1
                t = consts.tile(shape, dt, tag=f"c{_cid[0]}_{src_ap.tensor.name}")
                nc.sync.dma_start(t[:], src_ap)
                return t

            W_t = {}
            for nm, tt in (("k", t_Wk), ("q", t_Wq), ("v", t_Wv), ("s", t_Ws)):
                for l in range(NLAYER):
                    W_t[nm, l] = load_const(tt[l], [H, H], DT)
            We_t = [load_const(t_We[l], [128, H], DT) for l in range(NLAYER)]
            bias_qv_t = [load_const(t_bias_qv[l], [128, 2 * H], F32)
                         for l in range(NLAYER)] if meta["has_bias_qv"] else None
            bias_k_t = [load_const(t_bias_k[l], [128, H], F32)
                        for l in range(NLAYER)] if meta["has_bias_k"] else None
            bs_t = [load_const(t_bs[l], [H, 1], F32) for l in range(NLAYER)]
            A_t = [load_const(t_A[l], [H, 1], F32) for l in range(NLAYER)]
            B_t = [load_const(t_B[l], [H, 1], F32) for l in range(NLAYER)]
            iota_t = load_const(t_iota, [128, 128], DT)
            ident = consts.tile([128, 128], DT)
            make_identity(nc, ident[:])
            if use_bf16:
                identf = consts.tile([128, 128], F32)
                make_identity(nc, identf[:])
            else:
                identf = ident
            gidx_t = [load_const(t_gidx[0], [128, NGRP * T0 * 8], I16),
                      load_const(t_gidx[1], [128, NGRP * T1 * 8], I16)]
            kidx_t = [load_const(t_kidx[0], [128, NGRP * T0 * 8], I16),
                      load_const(t_kidx[1], [128, NGRP * T1 * 8], I16)]
            dst_t = [load_const(t_dst[0], [128, NGRP * T0], DT),
                     load_const(t_dst[1], [128, NGRP * T1], DT)]
            attr_t = [load_const(t_attr[0], [128, NGRP * T0], DT),
                      load_const(t_attr[1], [128, NGRP * T1], DT)]
            clin_t = load_const(t_clinT, [NCLIN, G], F32)
            Wch_t = load_const(t_Wc_h, [H, NCLS], F32)
            Wcc_t = load_const(t_Wc_c, [NCLIN, NCLS], F32)
            bc_t = load_const(t_bc, [G, NCLS], F32) if meta["has_bc"] else None

            hs = hsb.tile([128, NPAD], F32)        # s + agg accumulator
            h3_prev = None
            h3f = None
            copy_flip = 0

            for l in range(NLAYER):
                hsrc = t_x_rb if l == 0 else ag_out[l - 1]

                # ---- s-table (feature-major) into hs + local k table (node-major)
                for (c0, csz) in chunks:
                    if l == 0:
                        rhs_t = lhp.tile([128, csz], DT, tag="lh")
                        nc.sync.dma_start(rhs_t[:], t_xT_loc[:, c0:c0 + csz])
                        rhs_ap = rhs_t[:]
                    else:
                        rhs_ap = h3_prev[:, c0:c0 + csz]
                    ps = pnode.tile([128, csz], F32, tag="pn")
                    nc.tensor.matmul(out=ps[:], lhsT=W_t["s", l][:], rhs=rhs_ap,
                                     start=True, stop=True)
                    if meta["has_bs"]:
                        nc.scalar.activation(hs[:, c0:c0 + csz], ps[:], AF.Identity,
                                             bias=bs_t[l][:], scale=1.0)
                    else:
                        nc.scalar.activation(hs[:, c0:c0 + csz], ps[:], AF.Copy)
                    for s in range(csz // 128):
                        psk = pnode.tile([128, H], F32, tag="pn")
                        nc.tensor.matmul(out=psk[:],
                                         lhsT=rhs_ap[:, s * 128:(s + 1) * 128],
                                         rhs=W_t["k", l][:], start=True, stop=True)
                        stk = stg.tile([128, H], DT, tag="stk")
                        if meta["has_bias_k"]:
                            nc.vector.tensor_tensor(out=stk[:], in0=psk[:],
                                                    in1=bias_k_t[l][:], op=OP.add)
                        else:
                            nc.scalar.activation(stk[:], psk[:], AF.Copy)
                        nc.sync.dma_start(
                            k_loc[c0 + s * 128:c0 + (s + 1) * 128, :], stk[:])

                # ---- qv node tables (replicated over all ranks)
                for rb in range(R):
                    for (c0, csz) in chunks:
                        lh = lhp.tile([128, csz], DT, tag="lh")
                        nc.sync.dma_start(
                            lh[:], hsrc[rb * 128:(rb + 1) * 128, c0:c0 + csz])
                        for s in range(csz // 128):
                            row = rb * NPAD + c0 + s * 128
                            ps = pnode.tile([128, 2 * H], F32, tag="pn")
                            for j, nm in enumerate(("q", "v")):
                                nc.tensor.matmul(
                                    out=ps[:, j * H:(j + 1) * H],
                                    lhsT=lh[:, s * 128:(s + 1) * 128],
                                    rhs=W_t[nm, l][:], start=True, stop=True)
                            st = stg.tile([128, 2 * H], DT, tag="st")
                            if meta["has_bias_qv"]:
                                nc.vector.tensor_tensor(
                                    out=st[:], in0=ps[:], in1=bias_qv_t[l][:],
                                    op=OP.add)
                            else:
                                nc.scalar.activation(st[:], ps[:], AF.Copy)
                            if row < SPLIT:
                                nc.sync.dma_start(qv_lo[row:row + 128, :], st[:])
                            else:
                                nc.sync.dma_start(
                                    qv_hi[row - SPLIT:row - SPLIT + 128, :], st[:])

                # ---- edge phase
                for ep, T in ((0, T0), (1, T1)) if parts >= 2 else ():
                    qv_tab = qv_lo if ep == 0 else qv_hi
                    for g in range(NGRP):
                        nt = T
                        ne = nt * 128
                        isl = slice(g * T * 8, (g + 1) * T * 8)
                        gt = edg.tile([128, nt, 2 * H], DT, tag="g")
                        nc.gpsimd.dma_gather(
                            gt[:], qv_tab[:], gidx_t[ep][:, isl],
                            ne, ne, 2 * H, single_packet=(ne <= 512))
                        kt = edg.tile([128, nt, H], DT, tag="kt")
                        nc.gpsimd.dma_gather(
                            kt[:], k_loc[:], kidx_t[ep][:, isl],
                            ne, ne, H, single_packet=(ne <= 512))
                        S = edm.tile([128, nt, 128], DT, tag="S")
                        dsl = dst_t[ep][:, g * T:(g + 1) * T]
                        nc.vector.tensor_tensor(
                            out=S[:],
                            in0=dsl.unsqueeze(2).to_broadcast([128, nt, 128]),
                            in1=iota_t[:].unsqueeze(1).to_broadcast([128, nt, 128]),
                            op=OP.is_equal)
                        nc.vector.tensor_tensor(out=kt[:], in0=kt[:],
                                                in1=gt[:, :, 0:H], op=OP.add)
                        et = edm.tile([128, nt, H], DT, tag="et")
                        asl = attr_t[ep][:, g * T:(g + 1) * T]
                        nc.vector.tensor_tensor(
                            out=et[:],
                            in0=asl.unsqueeze(2).to_broadcast([128, nt, H]),
                            in1=We_t[l][:].unsqueeze(1).to_broadcast([128, nt, H]),
                            op=OP.mult)
                        nc.vector.tensor_tensor(out=kt[:], in0=kt[:], in1=et[:],
                                                op=OP.add)
                        nc.scalar.activation(kt[:], kt[:], AF.Sigmoid)
                        msg = edm.tile([128, nt, H], DT, tag="et")
                        nc.vector.tensor_tensor(out=msg[:], in0=kt[:],
                                                in1=gt[:, :, H:2 * H], op=OP.mult)
                        pa = pedge.tile([128, 128], F32, tag="pa")
                        for t in range(nt):
                            nc.tensor.matmul(out=pa[:], lhsT=msg[:, t, :],
                                             rhs=S[:, t, :], start=(t == 0),
                                             stop=(t == nt - 1))
                        nc.vector.tensor_tensor(
                            out=hs[:, g * 128:(g + 1) * 128],
                            in0=hs[:, g * 128:(g + 1) * 128], in1=pa[:], op=OP.add)

                # ---- h update: leaky + BN
                if parts < 3:
                    break
                LCH = 896
                for lc0 in range(0, NPAD, LCH):
                    lsz = min(LCH, NPAD - lc0)
                    sl = slice(lc0, lc0 + lsz)
                    tmp = stg.tile([128, lsz], F32, tag="lrelu")
                    nc.vector.tensor_scalar_mul(tmp[:], hs[:, sl], SLOPE)
                    nc.vector.tensor_tensor(out=hs[:, sl], in0=hs[:, sl], in1=tmp[:],
                                            op=OP.max)
                if l < 2:
                    h3 = h3p.tile([128, NPAD], DT)
                    nc.scalar.activation(h3[:], hs[:], AF.Identity,
                                         bias=B_t[l][:], scale=A_t[l][:])
                    nc.sync.dma_start(h_loc[l][:], h3[:])
                    nc.gpsimd.collective_compute(
                        "AllGather", OP.bypass,
                        replica_groups=[list(range(R))],
                        ins=[h_loc[l][:]], outs=[ag_out[l][:]])
                    h3_prev = h3
                else:
                    h3f = hsb.tile([128, NPAD], DT)
                    nc.scalar.activation(h3f[:], hs[:], AF.Identity,
                                         bias=B_t[l][:], scale=A_t[l][:])

            if parts < 4:
                z_dbg = stg.tile([G, NCLS], F32, tag="zsb")
                nc.vector.tensor_copy(z_dbg[:], hs[0:G, 0:NCLS])
                nc.sync.dma_start(t_out[:], z_dbg[:])
            else:
                # ---- pooling
                pp = ppool.tile([G, H], F32)
                for c in range(NGRP):
                    trp = pedge.tile([128, 128], DT, tag="pa")
                    nc.tensor.transpose(out=trp[:], in_=h3f[:, c * 128:(c + 1) * 128],
                                        identity=ident[:])
                    hnode = stg.tile([128, 128], DT, tag="hnode")
                    nc.vector.tensor_copy(hnode[:], trp[:])
                    ind_t = stg.tile([128, G], DT, tag="ind")
                    nc.sync.dma_start(ind_t[:], t_IndT[c * 128:(c + 1) * 128, :])
                    nc.tensor.matmul(out=pp[:], lhsT=ind_t[:], rhs=hnode[:],
                                     start=(c == 0), stop=(c == NGRP - 1))
                pool_sb = stg.tile([G, H], F32, tag="poolsb")
                nc.vector.tensor_copy(pool_sb[:], pp[:])
                nc.sync.dma_start(pool_in[:], pool_sb[:])
                nc.gpsimd.collective_compute(
                    "AllGather", OP.bypass, replica_groups=[list(range(R))],
                    ins=[pool_in[:]], outs=[pool_out[:]])
                # sum the 8 partial pools
                pr = stg.tile([G, R, H], F32, tag="pr")
                nc.sync.dma_start(pr[:], pool_out[:].rearrange("(r g) h -> g r h", r=R))
                pooled = stg.tile([G, H], F32, tag="pooled")
                nc.vector.tensor_tensor(out=pooled[:], in0=pr[:, 0, :], in1=pr[:, 1, :],
                                        op=OP.add)
                for r in range(2, R):
                    nc.vector.tensor_tensor(out=pooled[:], in0=pooled[:],
                                            in1=pr[:, r, :], op=OP.add)
                # transpose pooled [G,H] -> [H,G]
                ptp = pedge.tile([H, G], F32, tag="pa")
                nc.tensor.transpose(out=ptp[:], in_=pooled[:], identity=identf[0:G, 0:G])
                pooledT = stg.tile([H, G], F32, tag="pooledT")
                nc.vector.tensor_copy(pooledT[:], ptp[:])
                zp = pedge.tile([G, NCLS], F32, tag="pa")
                nc.tensor.matmul(out=zp[:], lhsT=pooledT[:], rhs=Wch_t[:],
                                 start=True, stop=False)
                nc.tensor.matmul(out=zp[:], lhsT=clin_t[:], rhs=Wcc_t[:],
                                 start=False, stop=True)
                z_sb = stg.tile([G, NCLS], F32, tag="zsb")
                if meta["has_bc"]:
                    nc.vector.tensor_tensor(out=z_sb[:], in0=zp[:], in1=bc_t[:],
                                            op=OP.add)
                else:
                    nc.vector.tensor_copy(z_sb[:], zp[:])
                nc.sync.dma_start(t_out[:], z_sb[:])

    nc.compile()
    return nc


# ---------------------------------------------------------------------------

_CACHE = {}


def kernel(**inputs):
    in_maps, meta = prep(inputs)
    key = tuple(sorted(meta.items()))
    if key not in _CACHE:
        _CACHE[key] = build(meta)
    nc = _CACHE[key]
    res = run_bass_kernel_spmd(nc, in_maps, list(range(R)))
    return np.asarray(res.results[0]["out"], np.float32)


def kernel_profiled(**inputs):
    """Like kernel() but also returns (exec_time_ns, trace_path)."""
    in_maps, meta = prep(inputs)
    key = tuple(sorted(meta.items()))
    if key not in _CACHE:
        _CACHE[key] = build(meta)
    nc = _CACHE[key]
    res = run_bass_kernel_spmd(nc, in_maps, list(range(R)), trace=True)
    out = np.asarray(res.results[0]["out"], np.float32)
    trace_path = None
    if res.instructions_and_trace is not None:
        trace_path = res.instructions_and_trace[1]
    return out, res.exec_time_ns, trace_path


if __name__ == "__main__":
    pass

